# revision 21
# baseline (speedup 1.0000x reference)
"""Multi-head attention (B=2, S=2048, E=1024, H=16, causal) on 8 Trainium2 cores.

Sharding: data-parallel over batch (2) x tensor-parallel over heads (4 groups
of 4 heads). Core i handles batch i//4, heads 4*(i%4) .. 4*(i%4)+3.
Each core computes Q/K/V projections for its 256 channels, causal
flash-attention for its 4 heads, and a partial output projection
(contribution of its channels to all 1024 output features). Partials are
summed across the 4 cores of each batch group (host-side).

Key optimizations over the straightforward version:
- input DMAs split across three queues (sync/scalar for X, vector for W)
- diagonal-tile column restriction: scores/exp/mask/PV only touch the
  causally-valid q columns of diagonal k-tiles (ideal-causal PE/ACT work)
- merged output projection (K=256 accumulated in PSUM, one DVE add, bf16
  stores split across sync/gpsimd queues)
- bias adds on gpsimd to keep DVE free for the norm chain
"""
import numpy as np

import concourse.bass as bass
import concourse.tile as tile
from concourse import bacc, mybir
from concourse.bass_utils import run_bass_kernel_spmd

F32 = mybir.dt.float32
F32R = mybir.dt.float32r
BF16 = mybir.dt.bfloat16
import ml_dtypes
MM_DT = BF16
MM_NP = ml_dtypes.bfloat16
OUT_DT = BF16
ActF = mybir.ActivationFunctionType
Alu = mybir.AluOpType

B, S, E = 2, 2048, 1024
H, DH = 16, 64
NCORES, TPW = 8, 4          # 8 cores, 4-way tensor parallel per batch
HPC = H // TPW              # heads per core = 4
C = HPC * DH                # channels per core = 256
SCALE = 1.0 / 8.0           # 1/sqrt(DH)
VW = HPC * (DH + 1)         # V storage width per s-tile (ones col per head)
NST = S // 128              # 16 s-tiles of 128 rows
NQB = S // 512              # 4 q-blocks of 512
NEC = E // 128              # 8 e-chunks (contraction for projections)

_cache = {}


def _emit(nc, tc, causal):
    # ---- DRAM parameters ----
    xt_d = nc.dram_tensor("xt", [E, S], MM_DT, kind="ExternalInput").ap()
    wqt_d = nc.dram_tensor("wqt", [E, C], MM_DT, kind="ExternalInput").ap()
    wkt_d = nc.dram_tensor("wkt", [E, C], MM_DT, kind="ExternalInput").ap()
    wvt_d = nc.dram_tensor("wvt", [E, C], MM_DT, kind="ExternalInput").ap()
    wot_d = nc.dram_tensor("wot", [C, E], MM_DT, kind="ExternalInput").ap()
    bqk_d = nc.dram_tensor("bqk", [128, 4], F32, kind="ExternalInput").ap()
    bv_d = nc.dram_tensor("bv", [1, C], F32, kind="ExternalInput").ap()
    bo_d = nc.dram_tensor("bo", [1, E], F32, kind="ExternalInput").ap()
    ones_d = nc.dram_tensor("ones", [1, 128], F32, kind="ExternalInput").ap()
    onesv_d = nc.dram_tensor("onesv", [128, NST * HPC], F32, kind="ExternalInput").ap()
    out_d = nc.dram_tensor("out", [S, E], OUT_DT, kind="ExternalOutput").ap()

    ctxpool = tc.tile_pool

    with ctxpool(name="persist", bufs=1) as pp:
        # ---- persistent SBUF tensors ----
        xt_sb = pp.tile([128, NEC * S], MM_DT)       # X^T, e-chunk ec at cols [ec*S)
        wvt_sb = pp.tile([128, NEC * C], MM_DT)
        wot_sb = pp.tile([128, 2 * E], MM_DT)        # c-chunk cc at cols [cc*E)
        qt_sb = pp.tile([128, 2 * S], MM_DT)         # Q^T, d-tile t at cols [t*S)
        kt_sb = pp.tile([128, 2 * S], MM_DT)
        v_sb = pp.tile([128, NST * VW], MM_DT)       # V (+ones col per head)
        ot_sb = pp.tile([128, 2 * S], MM_DT)         # normalized attn out^T
        bqk_sb = pp.tile([128, 4], F32)
        bvb_sb = pp.tile([128, C], F32)             # bv broadcast to partitions
        bob_sb = pp.tile([128, E], F32)             # bo broadcast to partitions
        ones_r = pp.tile([1, 128], F32R)

        def emit_vproj(psum_pool, st, vtag="mps"):
            """Project V for s-tile st into v_sb (with per-head ones column)."""
            ps = psum_pool.tile([128, C], F32, tag=vtag, name=f"vp{st}")
            for ec in range(NEC):
                nc.tensor.matmul(
                    ps[:],
                    xt_sb[:, ec * S + st * 128: ec * S + st * 128 + 128],
                    wvt_sb[:, ec * C: (ec + 1) * C],
                    start=(ec == 0), stop=(ec == NEC - 1),
                    skip_group_check=True)
            dst = v_sb[:, st * VW: st * VW + VW].rearrange(
                "p (h x) -> p h x", h=HPC)[:, :, 0:DH]
            nc.vector.tensor_add(
                dst,
                ps[:].rearrange("p (h x) -> p h x", h=HPC),
                bvb_sb[:].rearrange("p (h x) -> p h x", h=HPC))

        with ctxpool(name="qkw", bufs=1) as qkw, \
             ctxpool(name="small", bufs=1) as sp:
            wqt_sb = qkw.tile([128, NEC * C], MM_DT)
            wkt_sb = qkw.tile([128, NEC * C], MM_DT)
            bv_row = sp.tile([1, C], F32R)
            bo_row = sp.tile([1, E], F32R)

            # ---- input DMAs, split across queues in consumption order:
            # X^T on sync+scalar rings, Wq^T/Wk^T (then Wv^T/Wo^T) on the
            # vector ring, small bias tensors up front on scalar ----
            onesb_sb = sp.tile([128, NST * HPC], F32)
            # weights: one big strided DMA per tensor (per-descriptor cost
            # dominates at 64KB, so 8 small loads are ~4x slower than one).
            # gpsimd issues no DMAs at all: a gpsimd SWDGE ring costs ~3us of
            # Q7 drain in the kernel epilogue.  Every DMA has ~0.6us of fixed
            # queue cost, so the tiny bias tensors go AFTER the sb0/sb1
            # inputs that gate the first projections.
            def w_load(eng, w_sb, w_d, nch):
                eng.dma_start(
                    out=w_sb[:].rearrange("p (a c) -> p a c", c=nch),
                    in_=w_d[:].rearrange("(a p) c -> p a c", p=128))
            xt3 = xt_sb[:].rearrange("p (a s) -> p a s", s=S)

            def x_load(sb_i):
                # two e-chunks per DMA (a DMA has ~0.6us fixed cost)
                for ep in range(NEC // 2):
                    eng = nc.sync if ep % 2 == 0 else nc.scalar
                    eng.dma_start(
                        out=xt3[:, 2 * ep:2 * ep + 2,
                                sb_i * 512:(sb_i + 1) * 512],
                        in_=xt_d[2 * ep * 128:(2 * ep + 2) * 128,
                                 sb_i * 512:(sb_i + 1) * 512].rearrange(
                                     "(a p) c -> p a c", p=128))
            w_load(nc.sync, wqt_sb, wqt_d, C)     # gates first Q matmul
            x_load(0)
            w_load(nc.scalar, wkt_sb, wkt_d, C)
            nc.scalar.dma_start(out=bqk_sb[:], in_=bqk_d[:])
            w_load(nc.sync, wvt_sb, wvt_d, C)
            nc.scalar.dma_start(out=ones_r[:], in_=ones_d[:].bitcast(F32R))
            nc.scalar.dma_start(out=bv_row[:], in_=bv_d[:].bitcast(F32R))
            nc.scalar.dma_start(out=onesb_sb[:], in_=onesv_d[:])
            x_load(1)
            nc.scalar.dma_start(out=bo_row[:], in_=bo_d[:].bitcast(F32R))
            x_load(2)
            x_load(3)
            w_load(nc.scalar, wot_sb, wot_d, E)

            # ==== phase B: Q^T/K^T projections (e-chunk outer, 8 live
            # accumulation groups; PE paced by the DMA stream) ====
            with ctxpool(name="proj_ps", bufs=4, space="PSUM") as proj_ps:
                # dt0: s-block outer, 2 live groups; Q/K for sb0 land early
                # so the first q-block of attention overlaps the rest.
                # Bias broadcasts are tucked between sb0 and V (their input
                # rows arrive late; they must not lead the in-order PE queue)
                for sb_i in range(NQB):
                    for pj, w_sb, o_sb, bcol in ((0, wqt_sb, qt_sb, 0),
                                                 (1, wkt_sb, kt_sb, 2)):
                        ps = proj_ps.tile([128, 512], F32, tag="pps",
                                          name=f"pp0_{pj}_{sb_i}")
                        for ec in range(NEC):
                            nc.tensor.matmul(
                                ps[:],
                                w_sb[:, ec * C: ec * C + 128],
                                xt_sb[:, ec * S + sb_i * 512:
                                      ec * S + sb_i * 512 + 512],
                                start=(ec == 0), stop=(ec == NEC - 1),
                                skip_group_check=True)
                        nc.vector.tensor_scalar_add(
                            o_sb[:, sb_i * 512: sb_i * 512 + 512],
                            ps[:], bqk_sb[:, bcol: bcol + 1])
                    if sb_i == 0:
                        # bv broadcast, then V for the first q-block (all of
                        # V when non-causal)
                        ps_bv = proj_ps.tile([128, C], F32, tag="pps")
                        nc.tensor.matmul(ps_bv[:], ones_r[0:1, 0:128],
                                         bv_row[:], start=True, stop=True)
                        nc.vector.tensor_copy(bvb_sb[:], ps_bv[:])
                        for st in range(4):
                            emit_vproj(proj_ps, st, vtag="pps")
                        # V ones columns via a strided DVE copy (a strided
                        # DMA here costs ~10us of descriptor generation)
                        v_ones_ap = v_sb[:].rearrange(
                            "p (n x) -> p n x", x=DH + 1)[:, :, DH:DH + 1]
                        nc.vector.tensor_copy(
                            v_ones_ap,
                            onesb_sb[:].rearrange("p (n x) -> p n x", x=1))
                    if sb_i == 1:
                        for eb in range(2):
                            ps_bo = proj_ps.tile([128, 512], F32, tag="pps",
                                                 name=f"bo{eb}")
                            nc.tensor.matmul(
                                ps_bo[:], ones_r[0:1, 0:128],
                                bo_row[0:1, eb * 512:(eb + 1) * 512],
                                start=True, stop=True)
                            nc.vector.tensor_copy(
                                bob_sb[:, eb * 512:(eb + 1) * 512], ps_bo[:])
                if not causal:
                    for st in range(4, NST):
                        emit_vproj(proj_ps, st, vtag="pps")

            # ==== phase C: attention (q-block outer, head inner) + out-proj ====
            with ctxpool(name="score_ps", bufs=2, space="PSUM") as score_ps, \
                 ctxpool(name="attn_ps", bufs=2, space="PSUM") as attn_ps, \
                 ctxpool(name="misc_ps", bufs=2, space="PSUM") as misc_ps, \
                 ctxpool(name="pt_pool", bufs=10) as pt_pool, \
                 ctxpool(name="rec_pool", bufs=4) as rec_pool, \
                 ctxpool(name="out_pool", bufs=8) as out_pool:
                pending = []    # deferred norm closures of the previous hp
                pending_f = []  # deferred filler closures: (tag, closure)

                def flush_pending():
                    while pending:
                        pending.pop(0)()

                def flush_one():
                    if pending_f:
                        pending_f.pop(0)[1]()

                def flush_fillers(k=None):
                    n = len(pending_f) if k is None else min(k, len(pending_f))
                    for _ in range(n):
                        flush_one()

                def emit_dt1_part(sb_i, pj):
                    # one second-d-tile Q or K projection block (spread as
                    # fillers across the early attention steps)
                    w_sb, o_sb, bcol = ((wqt_sb, qt_sb, 0),
                                        (wkt_sb, kt_sb, 2))[pj]
                    with tc.high_priority(offset=-1_000_000):
                        ps1 = misc_ps.tile([128, 512], F32, tag="mps",
                                           name=f"pp1_{pj}_{sb_i}")
                        for ec in range(NEC):
                            nc.tensor.matmul(
                                ps1[:],
                                w_sb[:, ec * C + 128: ec * C + 256],
                                xt_sb[:, ec * S + sb_i * 512:
                                      ec * S + sb_i * 512 + 512],
                                start=(ec == 0), stop=(ec == NEC - 1),
                                skip_group_check=True)
                        nc.vector.tensor_scalar_add(
                            o_sb[:, S + sb_i * 512: S + sb_i * 512 + 512],
                            ps1[:], bqk_sb[:, bcol + 1: bcol + 2])

                ot_half = {}
                store_eng = [nc.sync, nc.sync]

                def emit_outproj_st(qb, st, mode="full", last=False):
                    # out-projection for s-tile st. mode "full": both c-chunks
                    # accumulated in PSUM; "cc0"/"cc1": the two head-pair
                    # halves split so the last q-block's cc0 half can serve as
                    # PE filler during its hp1 attention steps.
                    with tc.high_priority(offset=0 if last else -1_000_000):
                        if mode == "cc1":
                            o_t = ot_half[st]
                        else:
                            o_t = out_pool.tile([128, E], OUT_DT, tag="ob",
                                                name=f"ot{st}")
                            ot_half[st] = o_t
                        for eb in range(2):
                            ps_f = misc_ps.tile([128, 512], F32, tag="mps",
                                                name=f"pg{st}{eb}{mode}")
                            if mode != "cc1":
                                nc.tensor.matmul(
                                    ps_f[:],
                                    ot_sb[:, st * 128: st * 128 + 128],
                                    wot_sb[:, eb * 512: eb * 512 + 512],
                                    start=True, stop=(mode == "cc0"),
                                    skip_group_check=True)
                            if mode != "cc0":
                                nc.tensor.matmul(
                                    ps_f[:],
                                    ot_sb[:, S + st * 128: S + st * 128 + 128],
                                    wot_sb[:, E + eb * 512: E + eb * 512 + 512],
                                    start=(mode == "cc1"), stop=True,
                                    skip_group_check=True)
                            if mode == "cc1":
                                nc.vector.tensor_add(
                                    o_t[:, eb * 512:(eb + 1) * 512], ps_f[:],
                                    o_t[:, eb * 512:(eb + 1) * 512])
                            else:
                                nc.vector.tensor_add(
                                    o_t[:, eb * 512:(eb + 1) * 512], ps_f[:],
                                    bob_sb[:, eb * 512:(eb + 1) * 512])
                        if mode != "cc0":
                            store_eng[st % 2].dma_start(
                                out=out_d[st * 128:(st + 1) * 128, :],
                                in_=o_t[:])

                def emit_vproj_filler(st):
                    with tc.high_priority(offset=-1_000_000):
                        emit_vproj(misc_ps, st)

                for qb in range(NQB):
                    nk = 4 * (qb + 1) if causal else NST
                    q0 = qb * 512
                    if qb == 0:
                        # second-d-tile projections drip-fed as fillers
                        for sb_i in range(NQB):
                            for pj in range(2):
                                pending_f.append(
                                    (("dt1", sb_i),
                                     lambda sb_i=sb_i, pj=pj:
                                     emit_dt1_part(sb_i, pj)))
                    for hp in range(2):   # head pair (2*hp, 2*hp+1), d-tile hp
                        t = hp
                        ps_os = [None, None]
                        if hp == 1:
                            # hp1 scores need the d-tile-1 Q/K of every
                            # s-block this q-block touches
                            need = (nk - 1) // 4
                            while any(tg[0] == "dt1" and tg[1] <= need
                                      for tg, _ in pending_f):
                                flush_one()

                        def emit_pv(kt_i, pt, col0, hp=hp, nk=nk):
                            if causal:
                                # the V tile for this k-step may still be a
                                # queued filler
                                while any(tg == ("vp", kt_i)
                                          for tg, _ in pending_f):
                                    flush_one()
                            if kt_i == 0:
                                for a in range(2):
                                    ps_os[a] = attn_ps.tile(
                                        [65, 512], F32, tag="po",
                                        name=f"po{qb}{hp}{a}")
                            for a in range(2):
                                h = 2 * hp + a
                                nc.tensor.matmul(
                                    ps_os[a][:, col0:512],
                                    v_sb[:, kt_i * VW + h * (DH + 1):
                                         kt_i * VW + h * (DH + 1) + DH + 1],
                                    pt[:, a * 512 + col0:(a + 1) * 512],
                                    start=(kt_i == 0), stop=(kt_i == nk - 1),
                                    skip_group_check=True)

                        pv_queue = []
                        for kt_i in range(nk):
                            off = kt_i * 128 - q0
                            col0 = max(0, off) if causal else 0
                            ps_s = score_ps.tile([128, 1024], F32, tag="sc",
                                                 name=f"sc{qb}{hp}{kt_i}")
                            pt = pt_pool.tile([128, 1024], MM_DT, tag="pt",
                                              name=f"pt{qb}{hp}{kt_i}")
                            # the two heads' score matmuls target different PE
                            # row-groups (rows 0-63 vs 64-127)
                            for a in range(2):
                                p0 = a * 64
                                nc.tensor.matmul(
                                    ps_s[:, a * 512 + col0:(a + 1) * 512],
                                    kt_sb[p0:p0 + 64,
                                          t * S + kt_i * 128: t * S + kt_i * 128 + 128],
                                    qt_sb[p0:p0 + 64,
                                          t * S + q0 + col0: t * S + q0 + 512],
                                    start=True, stop=True)
                            if col0 == 0:
                                nc.scalar.activation(pt[:], ps_s[:], ActF.Exp,
                                                     scale=SCALE)
                            else:
                                pt3 = pt[:].rearrange(
                                    "p (u q) -> p u q", u=2)[:, :, col0:512]
                                ps3 = ps_s[:].rearrange(
                                    "p (u q) -> p u q", u=2)[:, :, col0:512]
                                nc.scalar.activation(pt3, ps3, ActF.Exp,
                                                     scale=SCALE)
                            if causal and off >= 0:
                                # triangular mask only on the 128-col diagonal
                                # chunk (cols < col0 are never read)
                                sel = pt[:].rearrange(
                                    "p (u q) -> p u q", u=2)[:, :, col0:col0 + 128]
                                nc.gpsimd.affine_select(
                                    out=sel, in_=sel,
                                    compare_op=Alu.is_ge,
                                    fill=0.0, base=0,
                                    pattern=[[0, 2], [1, 128]],
                                    channel_multiplier=-1)
                            if kt_i == 0:
                                # previous hp's norms land here, after this
                                # hp's first scores/exp are in the stream
                                flush_pending()
                            if kt_i >= 2:
                                # drip fillers into the PE stream; drain the
                                # queue promptly so nothing lumps at the tail.
                                # Exception: the last hp holds its cc0
                                # closures for its ACT-paced final steps.
                                if qb == NQB - 1 and hp == 1:
                                    if kt_i >= 10:
                                        flush_fillers(1)
                                else:
                                    flush_fillers(3 if len(pending_f) > 4 else
                                                  2 if len(pending_f) > 2
                                                  else 1)
                            # defer this step's PV: gives the in-order PE
                            # stream slack to clear the norm chain
                            pv_queue.append((kt_i, pt, col0))
                            if len(pv_queue) > 2:
                                emit_pv(*pv_queue.pop(0))
                        while pv_queue:
                            emit_pv(*pv_queue.pop(0))

                        last_hp = (qb == NQB - 1 and hp == 1)

                        def norm(qb=qb, hp=hp, t=t, q0=q0, ps_os=ps_os,
                                 split=last_hp):
                            # phase-by-phase across both heads, first 128
                            # columns first on the final norm, so the tail
                            # out-projection starts as soon as possible
                            rs, ps_bs, bc = {}, {}, {}
                            for a in range(2):
                                h = 2 * hp + a
                                rs[a] = rec_pool.tile([1, 512], F32R, tag="rs",
                                                      name=f"rs{qb}{h}")
                                nc.vector.tensor_copy(rs[a][:],
                                                      ps_os[a][64:65, :])
                            for a in range(2):
                                h = 2 * hp + a
                                ps_bs[a] = misc_ps.tile([64, 512], F32,
                                                        tag="mps",
                                                        name=f"pb{qb}{h}")
                                nc.tensor.matmul(ps_bs[a][:], ones_r[0:1, 0:64],
                                                 rs[a][:], start=True,
                                                 stop=True)
                            for a in range(2):
                                h = 2 * hp + a
                                bc[a] = rec_pool.tile([64, 512], F32, tag="bc",
                                                      name=f"bc{qb}{h}")
                                nc.vector.reciprocal_approx_fast(bc[a][:],
                                                                 ps_bs[a][:])
                            chunks = (((0, 128), (128, 512)) if split
                                      else ((0, 512),))
                            for c0, c1 in chunks:
                                for a in range(2):
                                    p0 = a * 64
                                    nc.vector.tensor_mul(
                                        ot_sb[p0:p0 + 64,
                                              t * S + q0 + c0: t * S + q0 + c1],
                                        ps_os[a][0:64, c0:c1], bc[a][:, c0:c1])
                        pending.append(norm)
                        if hp == 0:
                            if causal and qb + 1 < NQB:
                                for st in range(4 * (qb + 1), 4 * (qb + 2)):
                                    pending_f.append(
                                        (("vp", st),
                                         lambda st=st: emit_vproj_filler(st)))
                            if qb == NQB - 1:
                                # cc0 half of the last q-block's out-proj:
                                # PE filler during its hp1 attention steps
                                for st in range(qb * 4, qb * 4 + 4):
                                    pending_f.append(
                                        (("opc0", st),
                                         lambda qb=qb, st=st:
                                         emit_outproj_st(qb, st, mode="cc0")))
                        else:
                            if qb < NQB - 1:
                                for st in range(qb * 4, qb * 4 + 4):
                                    pending_f.append(
                                        (("op", st),
                                         lambda qb=qb, st=st:
                                         emit_outproj_st(qb, st)))
                flush_pending()
                flush_fillers()
                for st in range((NQB - 1) * 4, NQB * 4):
                    emit_outproj_st(NQB - 1, st, mode="cc1", last=True)


def _build(causal):
    nc = bacc.Bacc("TRN2", target_bir_lowering=False, debug=False,
                   num_devices=NCORES)
    with tile.TileContext(nc) as tc:
        _emit(nc, tc, causal)
    nc.compile()
    return nc


def _shard_inputs(QKV, Wq, bq, Wk, bk, Wv, bv, Wo, bo):
    QKV = np.asarray(QKV, dtype=np.float32)
    Wq, Wk, Wv, Wo = (np.asarray(w, dtype=np.float32) for w in (Wq, Wk, Wv, Wo))
    bq, bk, bv, bo = (np.asarray(b_, dtype=np.float32) for b_ in (bq, bk, bv, bo))
    ones = np.ones((1, 128), dtype=np.float32)
    onesv = np.ones((128, NST * HPC), dtype=np.float32)
    in_maps = []
    for core in range(NCORES):
        b, g = divmod(core, TPW)
        cs = slice(g * C, (g + 1) * C)
        bqs, bks = bq[cs], bk[cs]
        bqk = np.stack([bqs[:128], bqs[128:], bks[:128], bks[128:]], axis=1)
        in_maps.append({
            "xt": np.ascontiguousarray(QKV[b].T).astype(MM_NP),
            "wqt": np.ascontiguousarray(Wq[cs, :].T).astype(MM_NP),
            "wkt": np.ascontiguousarray(Wk[cs, :].T).astype(MM_NP),
            "wvt": np.ascontiguousarray(Wv[cs, :].T).astype(MM_NP),
            "wot": np.ascontiguousarray(Wo[:, cs].T).astype(MM_NP),
            "bqk": np.ascontiguousarray(bqk),
            "bv": bv[cs].reshape(1, C).copy(),
            # host sums the 4 tensor-parallel partials per batch; only one
            # core per group contributes the output bias
            "bo": (bo if g == 0 else np.zeros_like(bo)).reshape(1, E).copy(),
            "ones": ones,
            "onesv": onesv,
        })
    return in_maps


def kernel(QKV, Wq, bq, Wk, bk, Wv, bv, Wo, bo, is_causal):
    causal = bool(int(np.asarray(is_causal)))
    if causal not in _cache:
        _cache[causal] = _build(causal)
    nc = _cache[causal]
    in_maps = _shard_inputs(QKV, Wq, bq, Wk, bk, Wv, bv, Wo, bo)
    res = run_bass_kernel_spmd(nc, in_maps, core_ids=list(range(NCORES)))
    out = np.empty((B, S, E), dtype=np.float32)
    for b in range(B):
        acc = res.results[TPW * b]["out"].astype(np.float32)
        for g in range(1, TPW):
            acc = acc + res.results[TPW * b + g]["out"].astype(np.float32)
        out[b] = acc
    return out


# revision 23
# speedup vs baseline: 1.0231x; 1.0231x over previous
"""Multi-head attention (B=2, S=2048, E=1024, H=16, causal) on 8 Trainium2 cores.

Sharding: data-parallel over batch (2) x tensor-parallel over heads (4 groups
of 4 heads). Core i handles batch i//4, heads 4*(i%4) .. 4*(i%4)+3.
Each core computes Q/K/V projections for its 256 channels, causal
flash-attention for its 4 heads, and a partial output projection
(contribution of its channels to all 1024 output features). Partials are
summed across the 4 cores of each batch group (host-side).

Key optimizations over the straightforward version:
- input DMAs split across three queues (sync/scalar for X, vector for W)
- diagonal-tile column restriction: scores/exp/mask/PV only touch the
  causally-valid q columns of diagonal k-tiles (ideal-causal PE/ACT work)
- merged output projection (K=256 accumulated in PSUM, one DVE add, bf16
  stores split across sync/gpsimd queues)
- bias adds on gpsimd to keep DVE free for the norm chain
"""
import numpy as np

import concourse.bass as bass
import concourse.tile as tile
from concourse import bacc, mybir
from concourse.bass_utils import run_bass_kernel_spmd

F32 = mybir.dt.float32
F32R = mybir.dt.float32r
BF16 = mybir.dt.bfloat16
import ml_dtypes
MM_DT = BF16
MM_NP = ml_dtypes.bfloat16
OUT_DT = BF16
ActF = mybir.ActivationFunctionType
Alu = mybir.AluOpType

B, S, E = 2, 2048, 1024
H, DH = 16, 64
NCORES, TPW = 8, 4          # 8 cores, 4-way tensor parallel per batch
HPC = H // TPW              # heads per core = 4
C = HPC * DH                # channels per core = 256
SCALE = 1.0 / 8.0           # 1/sqrt(DH)
VW = HPC * (DH + 1)         # V storage width per s-tile (ones col per head)
NST = S // 128              # 16 s-tiles of 128 rows
NQB = S // 512              # 4 q-blocks of 512
NEC = E // 128              # 8 e-chunks (contraction for projections)

_cache = {}


def _emit(nc, tc, causal):
    # ---- DRAM parameters ----
    xt_d = nc.dram_tensor("xt", [E, S], MM_DT, kind="ExternalInput").ap()
    wqt_d = nc.dram_tensor("wqt", [E, C], MM_DT, kind="ExternalInput").ap()
    wkt_d = nc.dram_tensor("wkt", [E, C], MM_DT, kind="ExternalInput").ap()
    wvt_d = nc.dram_tensor("wvt", [E, C], MM_DT, kind="ExternalInput").ap()
    wot_d = nc.dram_tensor("wot", [C, E], MM_DT, kind="ExternalInput").ap()
    bqk_d = nc.dram_tensor("bqk", [128, 4], F32, kind="ExternalInput").ap()
    bv_d = nc.dram_tensor("bv", [1, C], F32, kind="ExternalInput").ap()
    bo_d = nc.dram_tensor("bo", [1, E], F32, kind="ExternalInput").ap()
    ones_d = nc.dram_tensor("ones", [1, 128], F32, kind="ExternalInput").ap()
    onesv_d = nc.dram_tensor("onesv", [128, NST * HPC], F32, kind="ExternalInput").ap()
    out_d = nc.dram_tensor("out", [S, E], OUT_DT, kind="ExternalOutput").ap()

    ctxpool = tc.tile_pool

    with ctxpool(name="persist", bufs=1) as pp:
        # ---- persistent SBUF tensors ----
        xt_sb = pp.tile([128, NEC * S], MM_DT)       # X^T, e-chunk ec at cols [ec*S)
        wvt_sb = pp.tile([128, NEC * C], MM_DT)
        wot_sb = pp.tile([128, 2 * E], MM_DT)        # c-chunk cc at cols [cc*E)
        qt_sb = pp.tile([128, 2 * S], MM_DT)         # Q^T, d-tile t at cols [t*S)
        kt_sb = pp.tile([128, 2 * S], MM_DT)
        v_sb = pp.tile([128, NST * VW], MM_DT)       # V (+ones col per head)
        ot_sb = pp.tile([128, 2 * S], MM_DT)         # normalized attn out^T
        bqk_sb = pp.tile([128, 4], F32)
        bvb_sb = pp.tile([128, C], F32)             # bv broadcast to partitions
        bob_sb = pp.tile([128, E], F32)             # bo broadcast to partitions
        ones_r = pp.tile([1, 128], F32R)

        def emit_vproj(psum_pool, st, vtag="mps"):
            """Project V for s-tile st into v_sb (with per-head ones column)."""
            ps = psum_pool.tile([128, C], F32, tag=vtag, name=f"vp{st}")
            for ec in range(NEC):
                nc.tensor.matmul(
                    ps[:],
                    xt_sb[:, ec * S + st * 128: ec * S + st * 128 + 128],
                    wvt_sb[:, ec * C: (ec + 1) * C],
                    start=(ec == 0), stop=(ec == NEC - 1),
                    skip_group_check=True)
            dst = v_sb[:, st * VW: st * VW + VW].rearrange(
                "p (h x) -> p h x", h=HPC)[:, :, 0:DH]
            nc.vector.tensor_add(
                dst,
                ps[:].rearrange("p (h x) -> p h x", h=HPC),
                bvb_sb[:].rearrange("p (h x) -> p h x", h=HPC))

        with ctxpool(name="qkw", bufs=1) as qkw, \
             ctxpool(name="small", bufs=1) as sp:
            wqt_sb = qkw.tile([128, NEC * C], MM_DT)
            wkt_sb = qkw.tile([128, NEC * C], MM_DT)
            bv_row = sp.tile([1, C], F32R)
            bo_row = sp.tile([1, E], F32R)

            # ---- input DMAs, split across queues in consumption order:
            # X^T on sync+scalar rings, Wq^T/Wk^T (then Wv^T/Wo^T) on the
            # vector ring, small bias tensors up front on scalar ----
            onesb_sb = sp.tile([128, NST * HPC], F32)
            # weights: one big strided DMA per tensor (per-descriptor cost
            # dominates at 64KB, so 8 small loads are ~4x slower than one).
            # gpsimd issues no DMAs at all: a gpsimd SWDGE ring costs ~3us of
            # Q7 drain in the kernel epilogue.  Every DMA has ~0.6us of fixed
            # queue cost, so the tiny bias tensors go AFTER the sb0/sb1
            # inputs that gate the first projections.
            def w_load(eng, w_sb, w_d, nch):
                eng.dma_start(
                    out=w_sb[:].rearrange("p (a c) -> p a c", c=nch),
                    in_=w_d[:].rearrange("(a p) c -> p a c", p=128))
            xt3 = xt_sb[:].rearrange("p (a s) -> p a s", s=S)

            def x_load(sb_i):
                # two e-chunks per DMA (a DMA has ~0.6us fixed cost)
                for ep in range(NEC // 2):
                    eng = nc.sync if ep % 2 == 0 else nc.scalar
                    eng.dma_start(
                        out=xt3[:, 2 * ep:2 * ep + 2,
                                sb_i * 512:(sb_i + 1) * 512],
                        in_=xt_d[2 * ep * 128:(2 * ep + 2) * 128,
                                 sb_i * 512:(sb_i + 1) * 512].rearrange(
                                     "(a p) c -> p a c", p=128))
            w_load(nc.sync, wqt_sb, wqt_d, C)     # gates first Q matmul
            w_load(nc.scalar, wkt_sb, wkt_d, C)
            x_load(0)
            x_load(1)
            nc.scalar.dma_start(out=bqk_sb[:], in_=bqk_d[:])
            w_load(nc.sync, wvt_sb, wvt_d, C)
            nc.scalar.dma_start(out=ones_r[:], in_=ones_d[:].bitcast(F32R))
            nc.scalar.dma_start(out=bv_row[:], in_=bv_d[:].bitcast(F32R))
            nc.scalar.dma_start(out=onesb_sb[:], in_=onesv_d[:])
            x_load(2)
            nc.scalar.dma_start(out=bo_row[:], in_=bo_d[:].bitcast(F32R))
            x_load(3)
            w_load(nc.scalar, wot_sb, wot_d, E)

            # ==== phase B: Q^T/K^T projections (e-chunk outer, 8 live
            # accumulation groups; PE paced by the DMA stream) ====
            with ctxpool(name="proj_ps", bufs=4, space="PSUM") as proj_ps:
                # dt0: s-block outer, 2 live groups; Q/K for sb0 land early
                # so the first q-block of attention overlaps the rest.
                # Bias broadcasts are tucked between sb0 and V (their input
                # rows arrive late; they must not lead the in-order PE queue)
                for sb_i in range(NQB):
                    for pj, w_sb, o_sb, bcol in ((0, wqt_sb, qt_sb, 0),
                                                 (1, wkt_sb, kt_sb, 2)):
                        ps = proj_ps.tile([128, 512], F32, tag="pps",
                                          name=f"pp0_{pj}_{sb_i}")
                        for ec in range(NEC):
                            nc.tensor.matmul(
                                ps[:],
                                w_sb[:, ec * C: ec * C + 128],
                                xt_sb[:, ec * S + sb_i * 512:
                                      ec * S + sb_i * 512 + 512],
                                start=(ec == 0), stop=(ec == NEC - 1),
                                skip_group_check=True)
                        nc.vector.tensor_scalar_add(
                            o_sb[:, sb_i * 512: sb_i * 512 + 512],
                            ps[:], bqk_sb[:, bcol: bcol + 1])
                    if sb_i == 1:
                        # bv broadcast, then V for the first q-block (all of
                        # V when non-causal); placed after sb1 so its Q/K
                        # do not queue behind a late wv load
                        ps_bv = proj_ps.tile([128, C], F32, tag="pps")
                        nc.tensor.matmul(ps_bv[:], ones_r[0:1, 0:128],
                                         bv_row[:], start=True, stop=True)
                        nc.vector.tensor_copy(bvb_sb[:], ps_bv[:])
                        for st in range(4):
                            emit_vproj(proj_ps, st, vtag="pps")
                        # V ones columns via a strided DVE copy (a strided
                        # DMA here costs ~10us of descriptor generation)
                        v_ones_ap = v_sb[:].rearrange(
                            "p (n x) -> p n x", x=DH + 1)[:, :, DH:DH + 1]
                        nc.vector.tensor_copy(
                            v_ones_ap,
                            onesb_sb[:].rearrange("p (n x) -> p n x", x=1))
                    if sb_i == 2:
                        for eb in range(2):
                            ps_bo = proj_ps.tile([128, 512], F32, tag="pps",
                                                 name=f"bo{eb}")
                            nc.tensor.matmul(
                                ps_bo[:], ones_r[0:1, 0:128],
                                bo_row[0:1, eb * 512:(eb + 1) * 512],
                                start=True, stop=True)
                            nc.vector.tensor_copy(
                                bob_sb[:, eb * 512:(eb + 1) * 512], ps_bo[:])
                if not causal:
                    for st in range(4, NST):
                        emit_vproj(proj_ps, st, vtag="pps")

            # ==== phase C: attention (q-block outer, head inner) + out-proj ====
            with ctxpool(name="score_ps", bufs=2, space="PSUM") as score_ps, \
                 ctxpool(name="attn_ps", bufs=2, space="PSUM") as attn_ps, \
                 ctxpool(name="misc_ps", bufs=2, space="PSUM") as misc_ps, \
                 ctxpool(name="pt_pool", bufs=10) as pt_pool, \
                 ctxpool(name="rec_pool", bufs=4) as rec_pool, \
                 ctxpool(name="out_pool", bufs=8) as out_pool:
                pending = []    # deferred norm closures of the previous hp
                pending_f = []  # deferred filler closures: (tag, closure)

                def flush_pending():
                    while pending:
                        pending.pop(0)()

                def flush_one():
                    if pending_f:
                        pending_f.pop(0)[1]()

                def flush_fillers(k=None):
                    n = len(pending_f) if k is None else min(k, len(pending_f))
                    for _ in range(n):
                        flush_one()

                def emit_dt1_part(sb_i, pj):
                    # one second-d-tile Q or K projection block (spread as
                    # fillers across the early attention steps)
                    w_sb, o_sb, bcol = ((wqt_sb, qt_sb, 0),
                                        (wkt_sb, kt_sb, 2))[pj]
                    with tc.high_priority(offset=-1_000_000):
                        ps1 = misc_ps.tile([128, 512], F32, tag="mps",
                                           name=f"pp1_{pj}_{sb_i}")
                        for ec in range(NEC):
                            nc.tensor.matmul(
                                ps1[:],
                                w_sb[:, ec * C + 128: ec * C + 256],
                                xt_sb[:, ec * S + sb_i * 512:
                                      ec * S + sb_i * 512 + 512],
                                start=(ec == 0), stop=(ec == NEC - 1),
                                skip_group_check=True)
                        nc.vector.tensor_scalar_add(
                            o_sb[:, S + sb_i * 512: S + sb_i * 512 + 512],
                            ps1[:], bqk_sb[:, bcol + 1: bcol + 2])

                ot_half = {}
                store_eng = [nc.sync, nc.sync]

                def emit_outproj_st(qb, st, mode="full", last=False):
                    # out-projection for s-tile st. mode "full": both c-chunks
                    # accumulated in PSUM; "cc0"/"cc1": the two head-pair
                    # halves split so the last q-block's cc0 half can serve as
                    # PE filler during its hp1 attention steps.
                    with tc.high_priority(offset=0 if last else -1_000_000):
                        if mode == "cc1":
                            o_t = ot_half[st]
                        else:
                            o_t = out_pool.tile([128, E], OUT_DT, tag="ob",
                                                name=f"ot{st}")
                            ot_half[st] = o_t
                        for eb in range(2):
                            ps_f = misc_ps.tile([128, 512], F32, tag="mps",
                                                name=f"pg{st}{eb}{mode}")
                            if mode != "cc1":
                                nc.tensor.matmul(
                                    ps_f[:],
                                    ot_sb[:, st * 128: st * 128 + 128],
                                    wot_sb[:, eb * 512: eb * 512 + 512],
                                    start=True, stop=(mode == "cc0"),
                                    skip_group_check=True)
                            if mode != "cc0":
                                nc.tensor.matmul(
                                    ps_f[:],
                                    ot_sb[:, S + st * 128: S + st * 128 + 128],
                                    wot_sb[:, E + eb * 512: E + eb * 512 + 512],
                                    start=(mode == "cc1"), stop=True,
                                    skip_group_check=True)
                            if mode == "cc1":
                                nc.vector.tensor_add(
                                    o_t[:, eb * 512:(eb + 1) * 512], ps_f[:],
                                    o_t[:, eb * 512:(eb + 1) * 512])
                            else:
                                nc.vector.tensor_add(
                                    o_t[:, eb * 512:(eb + 1) * 512], ps_f[:],
                                    bob_sb[:, eb * 512:(eb + 1) * 512])
                        if mode != "cc0":
                            store_eng[st % 2].dma_start(
                                out=out_d[st * 128:(st + 1) * 128, :],
                                in_=o_t[:])

                def emit_vproj_filler(st):
                    with tc.high_priority(offset=-1_000_000):
                        emit_vproj(misc_ps, st)

                for qb in range(NQB):
                    nk = 4 * (qb + 1) if causal else NST
                    q0 = qb * 512
                    if qb == 0:
                        # second-d-tile projections drip-fed as fillers
                        for sb_i in range(NQB):
                            for pj in range(2):
                                pending_f.append(
                                    (("dt1", sb_i),
                                     lambda sb_i=sb_i, pj=pj:
                                     emit_dt1_part(sb_i, pj)))
                    for hp in range(2):   # head pair (2*hp, 2*hp+1), d-tile hp
                        t = hp
                        ps_os = [None, None]
                        if hp == 1:
                            # hp1 scores need the d-tile-1 Q/K of every
                            # s-block this q-block touches
                            need = (nk - 1) // 4
                            while any(tg[0] == "dt1" and tg[1] <= need
                                      for tg, _ in pending_f):
                                flush_one()

                        def emit_pv(kt_i, pt, col0, hp=hp, nk=nk):
                            if causal:
                                # the V tile for this k-step may still be a
                                # queued filler
                                while any(tg == ("vp", kt_i)
                                          for tg, _ in pending_f):
                                    flush_one()
                            if kt_i == 0:
                                for a in range(2):
                                    ps_os[a] = attn_ps.tile(
                                        [65, 512], F32, tag="po",
                                        name=f"po{qb}{hp}{a}")
                            for a in range(2):
                                h = 2 * hp + a
                                nc.tensor.matmul(
                                    ps_os[a][:, col0:512],
                                    v_sb[:, kt_i * VW + h * (DH + 1):
                                         kt_i * VW + h * (DH + 1) + DH + 1],
                                    pt[:, a * 512 + col0:(a + 1) * 512],
                                    start=(kt_i == 0), stop=(kt_i == nk - 1),
                                    skip_group_check=True)

                        pv_queue = []
                        for kt_i in range(nk):
                            off = kt_i * 128 - q0
                            col0 = max(0, off) if causal else 0
                            ps_s = score_ps.tile([128, 1024], F32, tag="sc",
                                                 name=f"sc{qb}{hp}{kt_i}")
                            pt = pt_pool.tile([128, 1024], MM_DT, tag="pt",
                                              name=f"pt{qb}{hp}{kt_i}")
                            # the two heads' score matmuls target different PE
                            # row-groups (rows 0-63 vs 64-127)
                            for a in range(2):
                                p0 = a * 64
                                nc.tensor.matmul(
                                    ps_s[:, a * 512 + col0:(a + 1) * 512],
                                    kt_sb[p0:p0 + 64,
                                          t * S + kt_i * 128: t * S + kt_i * 128 + 128],
                                    qt_sb[p0:p0 + 64,
                                          t * S + q0 + col0: t * S + q0 + 512],
                                    start=True, stop=True)
                            if col0 == 0:
                                nc.scalar.activation(pt[:], ps_s[:], ActF.Exp,
                                                     scale=SCALE)
                            else:
                                pt3 = pt[:].rearrange(
                                    "p (u q) -> p u q", u=2)[:, :, col0:512]
                                ps3 = ps_s[:].rearrange(
                                    "p (u q) -> p u q", u=2)[:, :, col0:512]
                                nc.scalar.activation(pt3, ps3, ActF.Exp,
                                                     scale=SCALE)
                            if causal and off >= 0:
                                # triangular mask only on the 128-col diagonal
                                # chunk (cols < col0 are never read)
                                sel = pt[:].rearrange(
                                    "p (u q) -> p u q", u=2)[:, :, col0:col0 + 128]
                                nc.gpsimd.affine_select(
                                    out=sel, in_=sel,
                                    compare_op=Alu.is_ge,
                                    fill=0.0, base=0,
                                    pattern=[[0, 2], [1, 128]],
                                    channel_multiplier=-1)
                            if kt_i == 0:
                                # previous hp's norms land here, after this
                                # hp's first scores/exp are in the stream
                                flush_pending()
                            if kt_i >= 2:
                                # drip fillers into the PE stream; drain the
                                # queue promptly so nothing lumps at the tail
                                flush_fillers(3 if len(pending_f) > 4 else
                                              2 if len(pending_f) > 2 else 1)
                            # defer this step's PV: gives the in-order PE
                            # stream slack to clear the norm chain
                            pv_queue.append((kt_i, pt, col0))
                            if len(pv_queue) > 2:
                                emit_pv(*pv_queue.pop(0))
                        while pv_queue:
                            emit_pv(*pv_queue.pop(0))

                        last_hp = (qb == NQB - 1 and hp == 1)

                        def norm(qb=qb, hp=hp, t=t, q0=q0, ps_os=ps_os,
                                 split=last_hp):
                            for a in range(2):
                                h = 2 * hp + a
                                p0 = a * 64
                                rs = rec_pool.tile([1, 512], F32R, tag="rs",
                                                   name=f"rs{qb}{h}")
                                nc.vector.tensor_copy(rs[:], ps_os[a][64:65, :])
                                ps_b = misc_ps.tile([64, 512], F32, tag="mps",
                                                    name=f"pb{qb}{h}")
                                nc.tensor.matmul(ps_b[:], ones_r[0:1, 0:64],
                                                 rs[:], start=True, stop=True)
                                bc = rec_pool.tile([64, 512], F32, tag="bc",
                                                   name=f"bc{qb}{h}")
                                nc.vector.reciprocal_approx_fast(bc[:], ps_b[:])
                                for c0, c1 in (((0, 128), (128, 512))
                                               if split else ((0, 512),)):
                                    nc.vector.tensor_mul(
                                        ot_sb[p0:p0 + 64,
                                              t * S + q0 + c0: t * S + q0 + c1],
                                        ps_os[a][0:64, c0:c1], bc[:, c0:c1])
                        pending.append(norm)
                        if hp == 0:
                            if causal and qb + 1 < NQB:
                                for st in range(4 * (qb + 1), 4 * (qb + 2)):
                                    pending_f.append(
                                        (("vp", st),
                                         lambda st=st: emit_vproj_filler(st)))
                            if qb == NQB - 1:
                                # cc0 half of the last q-block's out-proj:
                                # PE filler during its hp1 attention steps
                                for st in range(qb * 4, qb * 4 + 4):
                                    pending_f.append(
                                        (("opc0", st),
                                         lambda qb=qb, st=st:
                                         emit_outproj_st(qb, st, mode="cc0")))
                        else:
                            if qb < NQB - 1:
                                for st in range(qb * 4, qb * 4 + 4):
                                    pending_f.append(
                                        (("op", st),
                                         lambda qb=qb, st=st:
                                         emit_outproj_st(qb, st)))
                flush_pending()
                flush_fillers()
                for st in range((NQB - 1) * 4, NQB * 4):
                    emit_outproj_st(NQB - 1, st, mode="cc1", last=True)


def _build(causal):
    nc = bacc.Bacc("TRN2", target_bir_lowering=False, debug=False,
                   num_devices=NCORES)
    with tile.TileContext(nc) as tc:
        _emit(nc, tc, causal)
    nc.compile()
    return nc


def _shard_inputs(QKV, Wq, bq, Wk, bk, Wv, bv, Wo, bo):
    QKV = np.asarray(QKV, dtype=np.float32)
    Wq, Wk, Wv, Wo = (np.asarray(w, dtype=np.float32) for w in (Wq, Wk, Wv, Wo))
    bq, bk, bv, bo = (np.asarray(b_, dtype=np.float32) for b_ in (bq, bk, bv, bo))
    ones = np.ones((1, 128), dtype=np.float32)
    onesv = np.ones((128, NST * HPC), dtype=np.float32)
    in_maps = []
    for core in range(NCORES):
        b, g = divmod(core, TPW)
        cs = slice(g * C, (g + 1) * C)
        bqs, bks = bq[cs], bk[cs]
        bqk = np.stack([bqs[:128], bqs[128:], bks[:128], bks[128:]], axis=1)
        in_maps.append({
            "xt": np.ascontiguousarray(QKV[b].T).astype(MM_NP),
            "wqt": np.ascontiguousarray(Wq[cs, :].T).astype(MM_NP),
            "wkt": np.ascontiguousarray(Wk[cs, :].T).astype(MM_NP),
            "wvt": np.ascontiguousarray(Wv[cs, :].T).astype(MM_NP),
            "wot": np.ascontiguousarray(Wo[:, cs].T).astype(MM_NP),
            "bqk": np.ascontiguousarray(bqk),
            "bv": bv[cs].reshape(1, C).copy(),
            # host sums the 4 tensor-parallel partials per batch; only one
            # core per group contributes the output bias
            "bo": (bo if g == 0 else np.zeros_like(bo)).reshape(1, E).copy(),
            "ones": ones,
            "onesv": onesv,
        })
    return in_maps


def kernel(QKV, Wq, bq, Wk, bk, Wv, bv, Wo, bo, is_causal):
    causal = bool(int(np.asarray(is_causal)))
    if causal not in _cache:
        _cache[causal] = _build(causal)
    nc = _cache[causal]
    in_maps = _shard_inputs(QKV, Wq, bq, Wk, bk, Wv, bv, Wo, bo)
    res = run_bass_kernel_spmd(nc, in_maps, core_ids=list(range(NCORES)))
    out = np.empty((B, S, E), dtype=np.float32)
    for b in range(B):
        acc = res.results[TPW * b]["out"].astype(np.float32)
        for g in range(1, TPW):
            acc = acc + res.results[TPW * b + g]["out"].astype(np.float32)
        out[b] = acc
    return out
